# revision 43
# baseline (speedup 1.0000x reference)
"""Additive attention kernel for Trainium2 (8 NeuronCores, SPMD).

Reference computation (B=4, L=1024, D=256, U=128):
    q = X @ W1                                   [B,L,U]
    k = X @ W2                                   [B,L,U]
    g = tanh(q[:,:,None,:] + k[:,None,:,:] + b1) [B,L,L,U]
    s = sigmoid(g @ W3 + b2)                     [B,L,L]
    out = s @ X                                  [B,L,D]

Sharding: 8 cores = (batch b, query-half h).  Each core handles 512 queries
against all 1024 keys of its batch.

Per-core dataflow (u=128 on SBUF partitions):
    qTb[u,q] = (W1^T X_q^T)[u,q] + b1[u]  fp16   (PE + ACT-identity)
    kT[u,k]  = (W2^T X_b^T)[u,k]          fp32   (PE)
    per key k:  gin[u,:] = qTb[u,:] + kT[u,k]    (DVE tensor_scalar, fp16)
    tanh in big fp16 [128, KB*512] tiles         (ACT - the bottleneck)
    psT[k%128, q] = sum_u W3[u]*gt[u,(k,q)]      (PE fp16, shifted-column Wd)
    scoreT[k,q] = sigmoid(psT + b2)              (ACT)
    out[q,d] += scoreT_kb^T @ X_kb               (PE fp32, accumulated per kb)
"""

import numpy as np

B, L, D, U = 4, 1024, 256, 128
QH = L // 2          # queries per core
KB = 32              # keys per tanh chunk (steady state)
N_CORES = 8

_CACHE = {}
LAST_RESULTS = None


def _build_program():
    import os
    import concourse.bass as bass
    import concourse.bacc as bacc
    import concourse.mybir as mybir
    import concourse.tile as tile
    from concourse import masks

    f32 = mybir.dt.float32
    f16 = mybir.dt.float16
    AF = mybir.ActivationFunctionType

    nc = bacc.Bacc(
        "TRN2",
        target_bir_lowering=False,
        debug=False,
        enable_asserts=False,
        num_devices=N_CORES,
    )

    Xb = nc.dram_tensor("Xb", [L, D], f32, kind="ExternalInput")
    Xq = nc.dram_tensor("Xq", [QH, D], f32, kind="ExternalInput")
    W1 = nc.dram_tensor("W1", [D, U], f32, kind="ExternalInput")
    W2 = nc.dram_tensor("W2", [D, U], f32, kind="ExternalInput")
    W3 = nc.dram_tensor("W3v", [U, 1], f32, kind="ExternalInput")
    b1 = nc.dram_tensor("b1", [U, 1], f32, kind="ExternalInput")
    b2 = nc.dram_tensor("b2", [1, 1], f32, kind="ExternalInput")
    out = nc.dram_tensor("out", [QH, D], f32, kind="ExternalOutput")

    NLB = L // 128   # 8 key blocks
    NQB = QH // 128  # 4 query blocks
    NDB = D // 128   # 2 d blocks

    # chunk sizes per key-block: ramp up in the first block so the first
    # tanh starts as early as possible; ramp down at the very end so the
    # final sigmoid/output dependency chain is short
    WARM = [4, 4, 8, 16, 32, 32, 32]
    COOL = [32, 32, 32, 16, 8, 8]
    assert sum(WARM) == 128 and sum(COOL) == 128

    with tile.TileContext(nc) as tc:
        with (
            tc.tile_pool(name="const", bufs=1) as cp,
            tc.tile_pool(name="ginA", bufs=1) as ginpA,
            tc.tile_pool(name="gtA", bufs=1) as gtpA,
            tc.tile_pool(name="ginB", bufs=1) as ginpB,
            tc.tile_pool(name="gtB", bufs=1) as gtpB,
            tc.tile_pool(name="score", bufs=2) as scp,
            tc.tile_pool(name="outs", bufs=2) as outp,
            tc.tile_pool(name="pre_ps", bufs=2, space="PSUM") as prepsum,
            tc.tile_pool(name="score_ps", bufs=2, space="PSUM") as scorepsum,
            tc.tile_pool(name="out_ps", bufs=1, space="PSUM") as outpsum,
        ):
            ident = cp.tile([128, 128], f32)
            masks.make_identity(nc, ident[:])

            # ---- load inputs (query chain first; spread across the two
            # HWDGE queues: sync + scalar) ----
            Xqs = cp.tile([128, NQB, D], f32)
            nc.sync.dma_start(
                Xqs[:, 0:NQB // 2, :],
                Xq[0:QH // 2].rearrange("(qb p) d -> p qb d", p=128))
            nc.scalar.dma_start(
                Xqs[:, NQB // 2:, :],
                Xq[QH // 2:QH].rearrange("(qb p) d -> p qb d", p=128))
            W1s = cp.tile([128, NDB, U], f32)
            nc.sync.dma_start(W1s[:], W1[:].rearrange("(db p) u -> p db u", p=128))
            b1s = cp.tile([128, 1], f32)
            nc.sync.dma_start(b1s[:], b1[:])
            W3s = cp.tile([128, 1], f32)
            nc.scalar.dma_start(W3s[:], W3[:])
            b2s = cp.tile([1, 1], f32)
            nc.scalar.dma_start(b2s[:], b2[:])
            W2s = cp.tile([128, NDB, U], f32)
            nc.scalar.dma_start(W2s[:], W2[:].rearrange("(db p) u -> p db u", p=128))
            # X_b natural (part = l%128); first key block in its own tile
            # so the early-kT chain doesn't wait for the full transfer
            XsA = cp.tile([128, 1, D], f32)
            nc.sync.dma_start(
                XsA[:], Xb[0:128].rearrange("(kb p) d -> p kb d", p=128))
            XsB = cp.tile([128, NLB // 2 - 1, D], f32)
            nc.sync.dma_start(
                XsB[:],
                Xb[128:L // 2].rearrange("(kb p) d -> p kb d", p=128))
            Xs1 = cp.tile([128, NLB // 2, D], f32)
            nc.scalar.dma_start(
                Xs1[:],
                Xb[L // 2:L].rearrange("(kb p) d -> p kb d", p=128))

            def xs_slice(kb):
                if kb == 0:
                    return XsA[:, 0, :]
                if kb < NLB // 2:
                    return XsB[:, kb - 1, :]
                return Xs1[:, kb - NLB // 2, :]

            ones1 = cp.tile([1, 128], f32)
            nc.vector.memset(ones1[:], 1.0)
            # Wd[:, 128] = W3, zero elsewhere.  Slicing Wd[:, 128-j:256-j]
            # yields a [128,128] stationary operand whose only nonzero
            # column is j - so matmul accumulates W3^T @ g into PSUM row j.
            Wd = cp.tile([128, 2 * 128], f16)
            nc.vector.memset(Wd[:], 0.0)
            nc.vector.tensor_copy(Wd[:, 128:129], W3s[:])

            # tile creation order is load-bearing (SBUF addresses feed the
            # DVE/ACT bank-interaction mode) - create first, emit after
            XqTs = cp.tile([128, NDB, QH], f32)    # X_q^T
            qTb = cp.tile([128, QH], f16)
            b2col = cp.tile([128, 1], f32)
            XT32 = cp.tile([128, NDB, 128], f32)
            kT32 = cp.tile([128, 32], f32)

            # ---- early kT for the first 32 keys (unblocks warmup chunks
            # while the rest of Xb is still being transposed) ----
            for db in range(NDB):
                tp = prepsum.tile([128, 128], f32, tag="pre")
                nc.tensor.transpose(
                    tp[:], XsA[:, 0, db * 128:(db + 1) * 128], ident[:]
                )
                nc.vector.tensor_copy(XT32[:, db, :], tp[:])
            kpre32 = prepsum.tile([128, 32], f32, tag="pre")
            for db in range(NDB):
                nc.tensor.matmul(
                    kpre32[:], W2s[:, db, :], XT32[:, db, 0:32],
                    start=(db == 0), stop=(db == NDB - 1),
                )
            nc.vector.tensor_copy(kT32[:], kpre32[:])

            # ---- transpose X_q (PE transpose via identity) ----
            for qb in range(NQB):
                for db in range(NDB):
                    tp = prepsum.tile([128, 128], f32, tag="pre")
                    nc.tensor.transpose(
                        tp[:], Xqs[:, qb, db * 128:(db + 1) * 128], ident[:]
                    )
                    nc.vector.tensor_copy(
                        XqTs[:, db, qb * 128:(qb + 1) * 128], tp[:]
                    )

            # ---- qTb[u,q] = W1^T Xq^T + b1  (fp16) ----
            qpre = prepsum.tile([128, QH], f32, tag="pre")
            for db in range(NDB):
                nc.tensor.matmul(
                    qpre[:], W1s[:, db, :], XqTs[:, db, :],
                    start=(db == 0), stop=(db == NDB - 1),
                )
            nc.scalar.activation(qTb[:], qpre[:], AF.Identity, bias=b1s[:])

            # ---- broadcast b2 across partitions: b2col = ones1.T @ b2s ----
            tpb = prepsum.tile([128, 1], f32, tag="pre")
            nc.tensor.matmul(tpb[:], ones1[:], b2s[:])
            nc.vector.tensor_copy(b2col[:], tpb[:])

            # ---- kT[u,k] = W2^T Xb^T, in two [128,512] tiles ----
            kT = []
            for lh in range(2):
                XTl = cp.tile([128, NDB, QH], f32, tag=f"XT{lh}",
                              name=f"XT{lh}")
                for lb in range(NLB // 2):
                    for db in range(NDB):
                        tp = prepsum.tile([128, 128], f32, tag="pre")
                        nc.tensor.transpose(
                            tp[:],
                            xs_slice(lh * (NLB // 2) + lb)[
                                :, db * 128:(db + 1) * 128],
                            ident[:]
                        )
                        nc.vector.tensor_copy(
                            XTl[:, db, lb * 128:(lb + 1) * 128], tp[:]
                        )
                kpre = prepsum.tile([128, QH], f32, tag="pre")
                for db in range(NDB):
                    nc.tensor.matmul(
                        kpre[:], W2s[:, db, :], XTl[:, db, :],
                        start=(db == 0), stop=(db == NDB - 1),
                    )
                kTl = cp.tile([128, QH], f32, tag=f"kT{lh}", name=f"kT{lh}")
                nc.vector.tensor_copy(kTl[:], kpre[:])
                kT.append(kTl)

            # address padding: the DVE<->ACT SBUF bank interaction is
            # sensitive to the absolute placement of the gin/gt pools;
            # this pad restores the empirically good congruence class
            pad = cp.tile([128, int(os.environ.get("PAD_F32", "3808"))], f32)
            nc.gpsimd.memset(pad[:, 0:1], 0.0)

            # ---- main loop over key blocks ----
            # one accumulator tile per query sub-block: each gets its own
            # PSUM bank (matmul start=True zeroes the WHOLE bank)
            po = [outpsum.tile([128, D], f32, tag=f"po{qs}", name=f"po{qs}")
                  for qs in range(NQB)]
            def emit_score(kb, psT):
                # sigmoid + fold this key block into the output accumulators
                scT = scp.tile([128, QH], f32, tag="scT", name="scT")
                nc.scalar.activation(scT[:], psT[:], AF.Sigmoid, bias=b2col[:])
                for qs in range(NQB):
                    nc.tensor.matmul(
                        po[qs][:], scT[:, qs * 128:(qs + 1) * 128],
                        xs_slice(kb),
                        start=(kb == 0), stop=(kb == NLB - 1),
                        skip_group_check=True,
                    )

            ci = 0  # global chunk counter for A/B buffer alternation
            pending = None  # (kb, psT) whose sigmoid is deferred one chunk
            for kb in range(NLB):
                psT = scorepsum.tile([128, QH], f32)
                if kb == 0:
                    sizes = WARM
                elif kb == NLB - 1:
                    sizes = COOL
                else:
                    sizes = [KB] * (128 // KB)
                kloc = 0
                for sz in sizes:
                    ginp = ginpA if ci % 2 == 0 else ginpB
                    gtp = gtpA if ci % 2 == 0 else gtpB
                    ci += 1
                    gin = ginp.tile([128, KB * QH], f16, tag="gin",
                                    name=f"gin{ci % 2}")
                    for j in range(sz):
                        k = kb * 128 + kloc + j
                        if k < 32:
                            kcol = kT32[:, k:k + 1]
                        else:
                            kcol = kT[k // QH][:, (k % QH):(k % QH) + 1]
                        nc.vector.tensor_scalar_add(
                            gin[:, j * QH:(j + 1) * QH], qTb[:], kcol,
                        )
                    gt = gtp.tile([128, KB * QH], f16, tag="gt",
                                  name=f"gt{ci % 2}")
                    nc.scalar.activation(
                        gt[:, :sz * QH], gin[:, :sz * QH], AF.Tanh
                    )
                    # previous block's sigmoid goes on the ACT queue right
                    # after this tanh, so ACT never stalls waiting for PE
                    if pending is not None:
                        emit_score(*pending)
                        pending = None
                    for j in range(sz):
                        nc.tensor.matmul(
                            psT[:], Wd[:, 128 - (kloc + j):256 - (kloc + j)],
                            gt[:, j * QH:(j + 1) * QH],
                            start=(kloc + j == 0), stop=(kloc + j == 127),
                        )
                    kloc += sz
                pending = (kb, psT)
            emit_score(*pending)

            # ---- write out (spread across both DMA queues) ----
            for qs in range(NQB):
                ot = outp.tile([128, D], f32, tag="ot", name="ot")
                nc.vector.tensor_copy(ot[:], po[qs][:])
                eng = nc.sync if qs % 2 == 0 else nc.scalar
                eng.dma_start(out[qs * 128:(qs + 1) * 128, :], ot[:])

    nc.compile()
    return nc


def _get_nc():
    if "nc" not in _CACHE:
        _CACHE["nc"] = _build_program()
    return _CACHE["nc"]


def kernel(X, W1, W2, W3, bias1, bias2, trace=False):
    global LAST_RESULTS
    from concourse.bass_utils import run_bass_kernel_spmd

    X = np.ascontiguousarray(np.asarray(X, dtype=np.float32))
    W1 = np.ascontiguousarray(np.asarray(W1, dtype=np.float32))
    W2 = np.ascontiguousarray(np.asarray(W2, dtype=np.float32))
    W3 = np.ascontiguousarray(np.asarray(W3, dtype=np.float32))
    b1 = np.ascontiguousarray(np.asarray(bias1, dtype=np.float32).reshape(U, 1))
    b2 = np.ascontiguousarray(np.asarray(bias2, dtype=np.float32).reshape(1, 1))

    nc = _get_nc()
    in_maps = []
    for c in range(N_CORES):
        b, h = c // 2, c % 2
        in_maps.append({
            "Xb": X[b],
            "Xq": np.ascontiguousarray(X[b, h * QH:(h + 1) * QH]),
            "W1": W1,
            "W2": W2,
            "W3v": W3,
            "b1": b1,
            "b2": b2,
        })

    res = run_bass_kernel_spmd(nc, in_maps, core_ids=list(range(N_CORES)),
                               trace=trace)
    LAST_RESULTS = res

    out = np.empty((B, L, D), dtype=np.float32)
    for c in range(N_CORES):
        b, h = c // 2, c % 2
        out[b, h * QH:(h + 1) * QH] = res.results[c]["out"]
    return out


# revision 44
# speedup vs baseline: 1.0056x; 1.0056x over previous
"""Additive attention kernel for Trainium2 (8 NeuronCores, SPMD).

Reference computation (B=4, L=1024, D=256, U=128):
    q = X @ W1                                   [B,L,U]
    k = X @ W2                                   [B,L,U]
    g = tanh(q[:,:,None,:] + k[:,None,:,:] + b1) [B,L,L,U]
    s = sigmoid(g @ W3 + b2)                     [B,L,L]
    out = s @ X                                  [B,L,D]

Sharding: 8 cores = (batch b, query-half h).  Each core handles 512 queries
against all 1024 keys of its batch.

Per-core dataflow (u=128 on SBUF partitions):
    qTb[u,q] = (W1^T X_q^T)[u,q] + b1[u]  fp16   (PE + ACT-identity)
    kT[u,k]  = (W2^T X_b^T)[u,k]          fp32   (PE)
    per key k:  gin[u,:] = qTb[u,:] + kT[u,k]    (DVE tensor_scalar, fp16)
    tanh in big fp16 [128, KB*512] tiles         (ACT - the bottleneck)
    psT[k%128, q] = sum_u W3[u]*gt[u,(k,q)]      (PE fp16, shifted-column Wd)
    scoreT[k,q] = sigmoid(psT + b2)              (ACT)
    out[q,d] += scoreT_kb^T @ X_kb               (PE fp32, accumulated per kb)
"""

import numpy as np

B, L, D, U = 4, 1024, 256, 128
QH = L // 2          # queries per core
KB = 32              # keys per tanh chunk (steady state)
N_CORES = 8

_CACHE = {}
LAST_RESULTS = None


def _build_program():
    import os
    import concourse.bass as bass
    import concourse.bacc as bacc
    import concourse.mybir as mybir
    import concourse.tile as tile
    from concourse import masks

    f32 = mybir.dt.float32
    f16 = mybir.dt.float16
    AF = mybir.ActivationFunctionType

    nc = bacc.Bacc(
        "TRN2",
        target_bir_lowering=False,
        debug=False,
        enable_asserts=False,
        num_devices=N_CORES,
    )

    Xb = nc.dram_tensor("Xb", [L, D], f32, kind="ExternalInput")
    Xq = nc.dram_tensor("Xq", [QH, D], f32, kind="ExternalInput")
    W1 = nc.dram_tensor("W1", [D, U], f32, kind="ExternalInput")
    W2 = nc.dram_tensor("W2", [D, U], f32, kind="ExternalInput")
    W3 = nc.dram_tensor("W3v", [U, 1], f32, kind="ExternalInput")
    b1 = nc.dram_tensor("b1", [U, 1], f32, kind="ExternalInput")
    b2 = nc.dram_tensor("b2", [1, 1], f32, kind="ExternalInput")
    out = nc.dram_tensor("out", [QH, D], f32, kind="ExternalOutput")

    NLB = L // 128   # 8 key blocks
    NQB = QH // 128  # 4 query blocks
    NDB = D // 128   # 2 d blocks

    # chunk sizes per key-block: ramp up in the first block so the first
    # tanh starts as early as possible; ramp down at the very end so the
    # final sigmoid/output dependency chain is short
    WARM = [4, 4, 8, 16, 32, 32, 32]
    COOL = [32, 32, 32, 16, 8, 8]
    assert sum(WARM) == 128 and sum(COOL) == 128

    with tile.TileContext(nc) as tc:
        with (
            tc.tile_pool(name="const", bufs=1) as cp,
            tc.tile_pool(name="ginA", bufs=1) as ginpA,
            tc.tile_pool(name="gtA", bufs=1) as gtpA,
            tc.tile_pool(name="ginB", bufs=1) as ginpB,
            tc.tile_pool(name="gtB", bufs=1) as gtpB,
            tc.tile_pool(name="score", bufs=2) as scp,
            tc.tile_pool(name="outs", bufs=2) as outp,
            tc.tile_pool(name="pre_ps", bufs=2, space="PSUM") as prepsum,
            tc.tile_pool(name="score_ps", bufs=2, space="PSUM") as scorepsum,
            tc.tile_pool(name="out_ps", bufs=1, space="PSUM") as outpsum,
        ):
            ident = cp.tile([128, 128], f32)
            masks.make_identity(nc, ident[:])

            # ---- load inputs (query chain first; spread across the two
            # HWDGE queues: sync + scalar) ----
            Xqs = cp.tile([128, NQB, D], f32)
            nc.sync.dma_start(
                Xqs[:, 0:NQB // 2, :],
                Xq[0:QH // 2].rearrange("(qb p) d -> p qb d", p=128))
            nc.scalar.dma_start(
                Xqs[:, NQB // 2:, :],
                Xq[QH // 2:QH].rearrange("(qb p) d -> p qb d", p=128))
            W1s = cp.tile([128, NDB, U], f32)
            nc.sync.dma_start(W1s[:], W1[:].rearrange("(db p) u -> p db u", p=128))
            b1s = cp.tile([128, 1], f32)
            nc.sync.dma_start(b1s[:], b1[:])
            W3s = cp.tile([128, 1], f32)
            nc.scalar.dma_start(W3s[:], W3[:])
            b2s = cp.tile([1, 1], f32)
            nc.scalar.dma_start(b2s[:], b2[:])
            W2s = cp.tile([128, NDB, U], f32)
            nc.scalar.dma_start(W2s[:], W2[:].rearrange("(db p) u -> p db u", p=128))
            # X_b natural (part = l%128); first key block in its own tile
            # so the early-kT chain doesn't wait for the full transfer
            XsA = cp.tile([128, 1, D], f32)
            nc.sync.dma_start(
                XsA[:], Xb[0:128].rearrange("(kb p) d -> p kb d", p=128))
            XsB = cp.tile([128, NLB // 2 - 1, D], f32)
            nc.sync.dma_start(
                XsB[:],
                Xb[128:L // 2].rearrange("(kb p) d -> p kb d", p=128))
            Xs1 = cp.tile([128, NLB // 2, D], f32)
            nc.scalar.dma_start(
                Xs1[:],
                Xb[L // 2:L].rearrange("(kb p) d -> p kb d", p=128))

            def xs_slice(kb):
                if kb == 0:
                    return XsA[:, 0, :]
                if kb < NLB // 2:
                    return XsB[:, kb - 1, :]
                return Xs1[:, kb - NLB // 2, :]

            ones1 = cp.tile([1, 128], f32)
            nc.vector.memset(ones1[:], 1.0)
            # Wd[:, 128] = W3, zero elsewhere.  Slicing Wd[:, 128-j:256-j]
            # yields a [128,128] stationary operand whose only nonzero
            # column is j - so matmul accumulates W3^T @ g into PSUM row j.
            Wd = cp.tile([128, 2 * 128], f16)
            nc.vector.memset(Wd[:], 0.0)
            nc.vector.tensor_copy(Wd[:, 128:129], W3s[:])

            # ---- transpose X_q then X_b (PE transpose via identity) ----
            XqTs = cp.tile([128, NDB, QH], f32)    # X_q^T
            for qb in range(NQB):
                for db in range(NDB):
                    tp = prepsum.tile([128, 128], f32, tag="pre")
                    nc.tensor.transpose(
                        tp[:], Xqs[:, qb, db * 128:(db + 1) * 128], ident[:]
                    )
                    nc.vector.tensor_copy(
                        XqTs[:, db, qb * 128:(qb + 1) * 128], tp[:]
                    )

            # ---- qTb[u,q] = W1^T Xq^T + b1  (fp16) ----
            qpre = prepsum.tile([128, QH], f32, tag="pre")
            for db in range(NDB):
                nc.tensor.matmul(
                    qpre[:], W1s[:, db, :], XqTs[:, db, :],
                    start=(db == 0), stop=(db == NDB - 1),
                )
            qTb = cp.tile([128, QH], f16)
            nc.scalar.activation(qTb[:], qpre[:], AF.Identity, bias=b1s[:])

            # ---- broadcast b2 across partitions: b2col = ones1.T @ b2s ----
            tpb = prepsum.tile([128, 1], f32, tag="pre")
            nc.tensor.matmul(tpb[:], ones1[:], b2s[:])
            b2col = cp.tile([128, 1], f32)
            nc.vector.tensor_copy(b2col[:], tpb[:])

            # ---- early kT for the first 32 keys (unblocks warmup chunks
            # while the rest of Xb is still being transposed) ----
            XT32 = cp.tile([128, NDB, 128], f32)
            for db in range(NDB):
                tp = prepsum.tile([128, 128], f32, tag="pre")
                nc.tensor.transpose(
                    tp[:], XsA[:, 0, db * 128:(db + 1) * 128], ident[:]
                )
                nc.vector.tensor_copy(XT32[:, db, :], tp[:])
            kpre32 = prepsum.tile([128, 32], f32, tag="pre")
            for db in range(NDB):
                nc.tensor.matmul(
                    kpre32[:], W2s[:, db, :], XT32[:, db, 0:32],
                    start=(db == 0), stop=(db == NDB - 1),
                )
            kT32 = cp.tile([128, 32], f32)
            nc.vector.tensor_copy(kT32[:], kpre32[:])

            # ---- kT[u,k] = W2^T Xb^T, in two [128,512] tiles ----
            kT = []
            for lh in range(2):
                XTl = cp.tile([128, NDB, QH], f32, tag=f"XT{lh}",
                              name=f"XT{lh}")
                for lb in range(NLB // 2):
                    for db in range(NDB):
                        tp = prepsum.tile([128, 128], f32, tag="pre")
                        nc.tensor.transpose(
                            tp[:],
                            xs_slice(lh * (NLB // 2) + lb)[
                                :, db * 128:(db + 1) * 128],
                            ident[:]
                        )
                        nc.vector.tensor_copy(
                            XTl[:, db, lb * 128:(lb + 1) * 128], tp[:]
                        )
                kpre = prepsum.tile([128, QH], f32, tag="pre")
                for db in range(NDB):
                    nc.tensor.matmul(
                        kpre[:], W2s[:, db, :], XTl[:, db, :],
                        start=(db == 0), stop=(db == NDB - 1),
                    )
                kTl = cp.tile([128, QH], f32, tag=f"kT{lh}", name=f"kT{lh}")
                nc.vector.tensor_copy(kTl[:], kpre[:])
                kT.append(kTl)

            # address padding: the DVE<->ACT SBUF bank interaction is
            # sensitive to the absolute placement of the gin/gt pools;
            # this pad restores the empirically good congruence class
            pad = cp.tile([128, int(os.environ.get("PAD_F32", "3808"))], f32)
            nc.gpsimd.memset(pad[:, 0:1], 0.0)

            # ---- main loop over key blocks ----
            # one accumulator tile per query sub-block: each gets its own
            # PSUM bank (matmul start=True zeroes the WHOLE bank)
            po = [outpsum.tile([128, D], f32, tag=f"po{qs}", name=f"po{qs}")
                  for qs in range(NQB)]
            def emit_score(kb, psT):
                # sigmoid + fold this key block into the output accumulators
                scT = scp.tile([128, QH], f32, tag="scT", name="scT")
                nc.scalar.activation(scT[:], psT[:], AF.Sigmoid, bias=b2col[:])
                for qs in range(NQB):
                    nc.tensor.matmul(
                        po[qs][:], scT[:, qs * 128:(qs + 1) * 128],
                        xs_slice(kb),
                        start=(kb == 0), stop=(kb == NLB - 1),
                        skip_group_check=True,
                    )

            ci = 0  # global chunk counter for A/B buffer alternation
            pending = None  # (kb, psT) whose sigmoid is deferred one chunk
            for kb in range(NLB):
                psT = scorepsum.tile([128, QH], f32)
                if kb == 0:
                    sizes = WARM
                elif kb == NLB - 1:
                    sizes = COOL
                else:
                    sizes = [KB] * (128 // KB)
                kloc = 0
                for sz in sizes:
                    ginp = ginpA if ci % 2 == 0 else ginpB
                    gtp = gtpA if ci % 2 == 0 else gtpB
                    ci += 1
                    gin = ginp.tile([128, KB * QH], f16, tag="gin",
                                    name=f"gin{ci % 2}")
                    for j in range(sz):
                        k = kb * 128 + kloc + j
                        if k < 32:
                            kcol = kT32[:, k:k + 1]
                        else:
                            kcol = kT[k // QH][:, (k % QH):(k % QH) + 1]
                        nc.vector.tensor_scalar_add(
                            gin[:, j * QH:(j + 1) * QH], qTb[:], kcol,
                        )
                    gt = gtp.tile([128, KB * QH], f16, tag="gt",
                                  name=f"gt{ci % 2}")
                    nc.scalar.activation(
                        gt[:, :sz * QH], gin[:, :sz * QH], AF.Tanh
                    )
                    # previous block's sigmoid goes on the ACT queue right
                    # after this tanh, so ACT never stalls waiting for PE
                    if pending is not None:
                        emit_score(*pending)
                        pending = None
                    for j in range(sz):
                        nc.tensor.matmul(
                            psT[:], Wd[:, 128 - (kloc + j):256 - (kloc + j)],
                            gt[:, j * QH:(j + 1) * QH],
                            start=(kloc + j == 0), stop=(kloc + j == 127),
                        )
                    kloc += sz
                pending = (kb, psT)
            emit_score(*pending)

            # ---- write out (spread across both DMA queues) ----
            for qs in range(NQB):
                ot = outp.tile([128, D], f32, tag="ot", name="ot")
                nc.vector.tensor_copy(ot[:], po[qs][:])
                eng = nc.sync if qs % 2 == 0 else nc.scalar
                eng.dma_start(out[qs * 128:(qs + 1) * 128, :], ot[:])

    nc.compile()
    return nc


def _get_nc():
    if "nc" not in _CACHE:
        _CACHE["nc"] = _build_program()
    return _CACHE["nc"]


def kernel(X, W1, W2, W3, bias1, bias2, trace=False):
    global LAST_RESULTS
    from concourse.bass_utils import run_bass_kernel_spmd

    X = np.ascontiguousarray(np.asarray(X, dtype=np.float32))
    W1 = np.ascontiguousarray(np.asarray(W1, dtype=np.float32))
    W2 = np.ascontiguousarray(np.asarray(W2, dtype=np.float32))
    W3 = np.ascontiguousarray(np.asarray(W3, dtype=np.float32))
    b1 = np.ascontiguousarray(np.asarray(bias1, dtype=np.float32).reshape(U, 1))
    b2 = np.ascontiguousarray(np.asarray(bias2, dtype=np.float32).reshape(1, 1))

    nc = _get_nc()
    in_maps = []
    for c in range(N_CORES):
        b, h = c // 2, c % 2
        in_maps.append({
            "Xb": X[b],
            "Xq": np.ascontiguousarray(X[b, h * QH:(h + 1) * QH]),
            "W1": W1,
            "W2": W2,
            "W3v": W3,
            "b1": b1,
            "b2": b2,
        })

    res = run_bass_kernel_spmd(nc, in_maps, core_ids=list(range(N_CORES)),
                               trace=trace)
    LAST_RESULTS = res

    out = np.empty((B, L, D), dtype=np.float32)
    for c in range(N_CORES):
        b, h = c // 2, c % 2
        out[b, h * QH:(h + 1) * QH] = res.results[c]["out"]
    return out
